# revision 1
# baseline (speedup 1.0000x reference)
"""Trainium2 Bass kernel for the 2-layer LSTMCell model.

Model (per timestep t, torch.nn.LSTMCell semantics, gates (i,f,g,o)):
    h0,c0 = LSTMCell(x_t, (h0,c0))   # D_IN=16  -> H1=100
    h1,c1 = LSTMCell(h0, (h1,c1))    # H1=100 -> H2=50
    y = h1_final @ W_fc.T + b_fc     # [B, 1]

Strategy (8 NeuronCores, data parallel over batch, 256 rows/core as
2 chunks of 128; the loop body is software-pipelined so layer-0's gate
matmuls for step t+1 are queued ahead of layer-1's for step t):

  - All matmul operands bf16; cell states bf16; PSUM f32 (except bf16
    transpose outputs).
  - Layer-0 gates i,f,g arrive batch-major ([128b, gates]) from per-chunk
    matmuls with the recurrent state A=[h0;1;x_t] ([117, 256]) stationary.
  - tanh(g) is computed as 2*sigmoid(2g)-1: the g-columns of the weights
    are pre-scaled by 2 on the host so ONE Sigmoid instruction covers
    i,f,g of both chunks; the 2x-1 fixup is a DVE tensor_scalar.
    Sigmoid outputs are stored gate-major so DVE slices are contiguous.
  - The layer-0 o gate is computed TRANSPOSED ([100h, 256b]) by a matmul
    with the weight block as stationary. c_new is transposed on the PE,
    tanh'd on ACT, and h0 = sig(o) * tanh(c) is then a [100,256] DVE mul
    that writes the next step's stationary tile directly -- no
    PSUM->SBUF copy on the recurrence path.
  - Layer 1 computes all four gates batch-major, takes tanh(c1) without
    a transpose, and transposes h1 itself; the PSUM->SBUF landing copy
    overlaps the next step.
  - x arrives via one staging DMA per 8 steps (partitions 96:116) and
    per-step GpSimd copies into the A tiles, keeping the Sync queue and
    DMA latency off the recurrence.
"""

import sys

import ml_dtypes
import numpy as np

BF = ml_dtypes.bfloat16

sys.path.insert(0, "/opt/trn_rl_repo")

import concourse.bacc as bacc
import concourse.bass as bass
import concourse.mybir as mybir
from concourse.tile import TileContext

F32 = mybir.dt.float32
BF16 = mybir.dt.bfloat16
Act = mybir.ActivationFunctionType
Alu = mybir.AluOpType

B, T, D_IN = 2048, 2048, 16
H1, H2 = 100, 50
N_CORES = 8
B_LOCAL = B // N_CORES        # 256
NCH = 2                       # chunks of 128 per core

# PE heater (disabled: real HW does not ramp past ~1.2GHz and the PE
# queue is already the bottleneck -- extra matmuls only serialize).
HEAT_A = 0
HEAT_B = 0
HEAT_C = 0
HEAT_COLS = 200  # must stay inside PSUM bank 0 of g0 (cols 300:500)

LAST_EXEC_NS = None

# ---------------------------------------------------------------- kernel build


def build_nc(t_steps=T):
    nk = t_steps // 8 + 1          # x chunks of 8 steps
    nc = bacc.Bacc("TRN2", target_bir_lowering=False)
    xt_d = nc.dram_tensor("xt", [nk, 21, 2048], BF16, kind="ExternalInput").ap()
    cb_d = nc.dram_tensor("cblob", [128, 929], BF16, kind="ExternalInput").ap()
    a0_d = nc.dram_tensor("a0", [117, 256], BF16, kind="ExternalInput").ap()
    y_d = nc.dram_tensor("y", [1, 256], F32, kind="ExternalOutput").ap()

    with TileContext(nc) as tc:
        with (
            tc.tile_pool(name="consts", bufs=1) as cp,
            tc.tile_pool(name="apool", bufs=8) as apool,
            tc.tile_pool(name="bpool", bufs=4) as bpool,
            tc.tile_pool(name="xstage", bufs=3) as xsp,
            tc.tile_pool(name="ew", bufs=2) as ew,
            tc.tile_pool(name="g0ps", bufs=1, space="PSUM") as g0pool,
            tc.tile_pool(name="g1ps", bufs=1, space="PSUM") as g1pool,
            tc.tile_pool(name="ops", bufs=1, space="PSUM") as opool,
            tc.tile_pool(name="tps", bufs=1, space="PSUM") as tpool,
        ):
            cb = cp.tile([128, 929], BF16)
            nc.sync.dma_start(cb, cb_d)
            w0 = cb[0:117, 0:400]        # cols: i,f,2g (0:300) | o (300:400)
            wih1 = cb[0:101, 400:600]    # cols: i,f,2g (0:150) | o (150:200)
            whh1 = cb[0:50, 600:800]
            wfcb = cb[0:51, 800:801]     # rows 0:50 = W_fc, row 50 = b_fc
            ident = cb[0:128, 801:929]

            # x staging: one DMA per 8 steps into partitions 100:116,
            # then per-step GpSimd copies into the A tiles.
            stages = {}

            def dma_stage(k):
                if k >= nk or k in stages:
                    return
                s = xsp.tile([117, 2048], BF16, tag="xs")
                nc.sync.dma_start(s[96:117, :], xt_d[k])
                stages[k] = s

            def xcopy(g):
                """GP-copy x for global step g into A_q[g] rows 100:117."""
                if g > t_steps:
                    return
                s = stages[g // 8]
                nc.gpsimd.tensor_copy(
                    A_q[g][96:117, :],
                    s[96:117, (g % 8) * 256 : (g % 8) * 256 + 256],
                )

            dma_stage(0)
            dma_stage(1)
            dma_stage(2)

            # initial state
            A_q = []
            a = apool.tile([117, 256], BF16, tag="A")
            nc.sync.dma_start(a, a0_d)
            A_q.append(a)
            a1 = apool.tile([117, 256], BF16, tag="A")
            A_q.append(a1)
            a2 = apool.tile([117, 256], BF16, tag="A")
            A_q.append(a2)
            xcopy(1)
            Btile = bpool.tile([50, 256], BF16, tag="B")
            nc.vector.memset(Btile[:, :], 0.0)
            c0 = ew.tile([128, 200], BF16, tag="c0")
            nc.vector.memset(c0[:, :], 0.0)
            c1 = ew.tile([128, 100], BF16, tag="c1")
            nc.vector.memset(c1[:, :], 0.0)

            def heat(n):
                for k in range(n):
                    nc.tensor.matmul(
                        g0_cur[0:1, 300 : 300 + HEAT_COLS],
                        ident[0:1, 0:1],
                        cb[0:1, 0:HEAT_COLS],
                        start=True, stop=True,
                    )

            def emit_g0_sig(t):
                """g0/oT matmuls + sigmoids for step t (reads A(t)).

                Sigmoid output S is stored gate-major: col = g*200 + c*100 + j
                so each gate slice is contiguous for the DVE ops."""
                A = A_q[t]
                g0 = g0pool.tile([128, 1024], F32, tag="g0")
                for c in range(NCH):
                    nc.tensor.matmul(
                        g0[:, c * 512 : c * 512 + 300],
                        A[:, c * 128 : (c + 1) * 128],
                        w0[:, 0:300],
                        start=True, stop=True,
                    )
                po = opool.tile([100, 256], F32, tag="po")
                nc.tensor.matmul(po, w0[:, 300:400], A, start=True, stop=True)
                g0v = g0.rearrange("p (c f) -> p c f", c=2)   # [128, 2, 512]
                g0v4 = g0v[:, :, 0:300].rearrange("p c (g j) -> p c g j", g=3)
                S = ew.tile([128, 600], BF16, tag="S")
                S4 = S.rearrange("p (g c j) -> p c g j", g=3, c=2)
                nc.scalar.activation(S4, g0v4, Act.Sigmoid)
                SoT = ew.tile([100, 256], BF16, tag="SoT")
                nc.scalar.activation(SoT, po, Act.Sigmoid)
                return g0, S, SoT

            # prologue: gates + sigmoids for step 0
            g0_cur, S_cur, SoT_cur = emit_g0_sig(0)
            ph1_prev = None

            for t in range(t_steps):
                S, SoT = S_cur, SoT_cur
                An = A_q[t + 1]

                # ---- GP: x for A(t+2), then forget-gate product
                xcopy(t + 2)
                m2 = ew.tile([128, 200], BF16, tag="m2")
                nc.gpsimd.tensor_tensor(m2, S[:, 200:400], c0, Alu.mult)

                # ---- DVE: L0 c update (bf16, contiguous)
                ts = ew.tile([128, 200], BF16, tag="ts")
                nc.vector.tensor_scalar(ts, S[:, 400:600], 2.0, 1.0,
                                        Alu.mult, Alu.subtract)
                Bn = bpool.tile([50, 256], BF16, tag="B")
                if ph1_prev is not None:
                    nc.vector.tensor_copy(Bn, ph1_prev)
                else:
                    nc.vector.memset(Bn[:, :], 0.0)
                Btile = Bn
                m1 = ew.tile([128, 200], BF16, tag="m1")
                nc.vector.tensor_tensor(m1, ts, S[:, 0:200], Alu.mult)
                c0n = ew.tile([128, 200], BF16, tag="c0")
                nc.vector.tensor_tensor(c0n, m1, m2, Alu.add)

                # ---- PE: c0 transpose, then heater for the tanh/h0 wait
                pc = tpool.tile([100, 256], BF16, tag="pc")
                for c in range(NCH):
                    nc.tensor.transpose(
                        pc[:, c * 128 : (c + 1) * 128],
                        c0n[:, c * 100 : (c + 1) * 100],
                        ident,
                    )
                heat(HEAT_A)

                # ---- ACT sig(2c) + DVE fixup = tanh(c); h0 into A(t+1)
                thc_s = ew.tile([100, 256], BF16, tag="thc_s")
                nc.scalar.activation(thc_s, pc, Act.Sigmoid, scale=2.0)
                thc = ew.tile([100, 256], BF16, tag="thc")
                nc.vector.tensor_scalar(thc, thc_s, 2.0, 1.0,
                                        Alu.mult, Alu.subtract)
                nc.vector.tensor_tensor(An[0:100, :], SoT, thc, Alu.mult)

                # ---- PE: g0(t+1)/oT(t+1) + sigmoids (pipelined ahead)
                if t + 1 < t_steps:
                    g0_cur, S_cur, SoT_cur = emit_g0_sig(t + 1)

                # ---- PE: g1(t) from A(t+1) rows 0:101 (= h0(t), ones);
                #      all four gates batch-major
                g1 = g1pool.tile([128, 512], F32, tag="g1")
                for c in range(NCH):
                    nc.tensor.matmul(
                        g1[:, c * 256 : c * 256 + 200],
                        An[0:101, c * 128 : (c + 1) * 128],
                        wih1,
                        start=True, stop=False,
                    )
                    nc.tensor.matmul(
                        g1[:, c * 256 : c * 256 + 200],
                        Btile[:, c * 128 : (c + 1) * 128],
                        whh1,
                        start=False, stop=True,
                    )
                heat(HEAT_B)

                # ---- ACT: L1 sigmoid (gate-major, all four gates)
                g1v = g1.rearrange("p (c f) -> p c f", c=2)   # [128, 2, 256]
                g1v4 = g1v[:, :, 0:200].rearrange("p c (g j) -> p c g j", g=4)
                S1 = ew.tile([128, 400], BF16, tag="S1")
                S14 = S1.rearrange("p (g c j) -> p c g j", g=4, c=2)
                nc.scalar.activation(S14, g1v4, Act.Sigmoid)

                # ---- GP: L1 forget product; DVE: L1 c update
                m3 = ew.tile([128, 100], BF16, tag="m3")
                nc.gpsimd.tensor_tensor(m3, S1[:, 100:200], c1, Alu.mult)
                ts1 = ew.tile([128, 100], BF16, tag="ts1")
                nc.vector.tensor_scalar(ts1, S1[:, 200:300], 2.0, 1.0,
                                        Alu.mult, Alu.subtract)
                m4 = ew.tile([128, 100], BF16, tag="m4")
                nc.vector.tensor_tensor(m4, S1[:, 0:100], ts1, Alu.mult)
                c1n = ew.tile([128, 100], BF16, tag="c1")
                nc.vector.tensor_tensor(c1n, m4, m3, Alu.add)

                # ---- ACT sig(2c1) + DVE fixup; DVE h1; PE h1 transpose
                thc1_s = ew.tile([128, 100], BF16, tag="thc1_s")
                nc.scalar.activation(thc1_s, c1n, Act.Sigmoid, scale=2.0)
                thc1 = ew.tile([128, 100], BF16, tag="thc1")
                nc.vector.tensor_scalar(thc1, thc1_s, 2.0, 1.0,
                                        Alu.mult, Alu.subtract)
                h1t = ew.tile([128, 100], BF16, tag="h1t")
                nc.vector.tensor_tensor(h1t, S1[:, 300:400], thc1, Alu.mult)
                ph1 = tpool.tile([50, 256], BF16, tag="pc1")
                for c in range(NCH):
                    nc.tensor.transpose(
                        ph1[:, c * 128 : (c + 1) * 128],
                        h1t[:, c * 50 : (c + 1) * 50],
                        ident,
                    )
                heat(HEAT_C)
                ph1_prev = ph1

                # ---- x staging: allocate A(t+3), stage DMA, copy x to A(t+2)
                if t + 3 <= t_steps:
                    a_next = apool.tile([117, 256], BF16, tag="A")
                    A_q.append(a_next)
                if t % 8 == 0:
                    dma_stage(t // 8 + 2)

                c0, c1 = c0n, c1n

            # ---- epilogue: final h1, then y = h1 @ W_fc.T + b_fc
            Blast = bpool.tile([50, 256], BF16, tag="B")
            nc.vector.tensor_copy(Blast, ph1_prev)
            Btile = Blast
            fin = ew.tile([51, 256], BF16, tag="fin")
            nc.vector.memset(fin[:, :], 1.0)
            nc.vector.tensor_copy(fin[0:50, :], Btile)
            yp = g1pool.tile([1, 256], F32, tag="yp")
            nc.tensor.matmul(yp, wfcb, fin, start=True, stop=True)
            ysb = ew.tile([1, 256], F32, tag="ysb")
            nc.scalar.copy(ysb, yp)
            nc.sync.dma_start(y_d, ysb)
    return nc


# ---------------------------------------------------------------- host prep


def _pack_gates(w, h):
    """[4h, d] torch-order (i,f,g,o) -> [d, 4h] columns (i, f, 2g, o)."""
    wt = np.asarray(w, np.float32).T if w.ndim == 2 else np.asarray(w, np.float32)[None, :]
    i, f, g, o = wt[:, 0:h], wt[:, h:2*h], wt[:, 2*h:3*h], wt[:, 3*h:4*h]
    return np.concatenate([i, f, 2.0 * g, o], axis=1)


def prep_weights(W_ih0, W_hh0, b_ih0, b_hh0, W_ih1, W_hh1, b_ih1, b_hh1, W_fc, b_fc):
    cb = np.zeros((128, 929), np.float32)
    cb[0:100, 0:400] = _pack_gates(W_hh0, H1)
    cb[100, 0:400] = _pack_gates(np.asarray(b_ih0) + np.asarray(b_hh0), H1)[0]
    cb[101:117, 0:400] = _pack_gates(W_ih0, H1)
    cb[0:100, 400:600] = _pack_gates(W_ih1, H2)
    cb[100, 400:600] = _pack_gates(np.asarray(b_ih1) + np.asarray(b_hh1), H2)[0]
    cb[0:50, 600:800] = _pack_gates(W_hh1, H2)
    cb[0:50, 800] = np.asarray(W_fc, np.float32)[0]
    cb[50, 800] = float(np.asarray(b_fc).reshape(-1)[0])
    cb[:, 801:929] = np.eye(128, dtype=np.float32)
    return cb.astype(BF)


def prep_x_core(x_core, t_steps):
    """x_core [256, T, 16] -> bf16 [nk, 21, 8*256] chunks of 8 steps.

    Row r of a slot maps to A-tile partition 96+r: rows 0:4 are scratch
    (overwritten by the h0 write), row 4 is the ones row, rows 5:21 are
    x_t.T (slot t_steps and beyond: ones + zeros)."""
    nk = t_steps // 8 + 1
    tmp = np.zeros((nk * 8, 21, 256), BF)
    tmp[:, 4, :] = np.asarray(1.0, BF)
    tmp[:t_steps, 5:21, :] = (
        np.asarray(x_core, np.float32).transpose(1, 2, 0).astype(BF))
    return tmp.reshape(nk, 8, 21, 256).transpose(0, 2, 1, 3).reshape(nk, 21, 2048)


_RUNNER_CACHE = {}


def _get_runner(t_steps):
    if t_steps in _RUNNER_CACHE:
        return _RUNNER_CACHE[t_steps]

    import jax
    from jax.experimental.shard_map import shard_map
    from jax.sharding import Mesh, NamedSharding, PartitionSpec

    from concourse import bass2jax

    bass2jax.install_neuronx_cc_hook()
    nc = build_nc(t_steps)
    if not nc.is_finalized():
        nc.finalize()
    global _LAST_NC
    _LAST_NC = nc

    partition_name = (
        nc.partition_id_tensor.name if nc.partition_id_tensor else None
    )
    in_names = []
    out_names = []
    out_avals = []
    zero_outs = []
    for alloc in nc.m.functions[0].allocations:
        if not isinstance(alloc, mybir.MemoryLocationSet):
            continue
        name = alloc.memorylocations[0].name
        if alloc.kind == "ExternalInput":
            if name == partition_name:
                continue
            in_names.append(name)
        elif alloc.kind == "ExternalOutput":
            out_names.append(name)
            shape = tuple(alloc.tensor_shape)
            dtype = mybir.dt.np(alloc.dtype)
            out_avals.append(jax.core.ShapedArray(shape, dtype))
            zero_outs.append(np.zeros(shape, dtype))
    n_params = len(in_names)
    all_in_names = in_names + out_names
    if partition_name is not None:
        all_in_names = all_in_names + [partition_name]

    def _body(*args):
        operands = list(args)
        if partition_name is not None:
            operands.append(bass2jax.partition_id_tensor())
        outs = bass2jax._bass_exec_p.bind(
            *operands,
            out_avals=tuple(out_avals),
            in_names=tuple(all_in_names),
            out_names=tuple(out_names),
            lowering_input_output_aliases=(),
            sim_require_finite=True,
            sim_require_nnan=True,
            nc=nc,
        )
        return tuple(outs)

    devices = jax.devices()[:N_CORES]
    mesh = Mesh(np.asarray(devices), ("core",))
    spec = PartitionSpec("core")
    in_specs = (spec,) * (n_params + len(out_names))
    out_specs = (spec,) * len(out_names)
    sharded = jax.jit(
        shard_map(_body, mesh=mesh, in_specs=in_specs, out_specs=out_specs,
                  check_rep=False),
        keep_unused=True,
    )
    sharding = NamedSharding(mesh, spec)

    def run(concat_inputs, n_bench=0):
        import time as _time

        args = [jax.device_put(concat_inputs[n], sharding) for n in in_names]
        args += [jax.device_put(
            np.zeros((N_CORES * z.shape[0], *z.shape[1:]), z.dtype), sharding)
            for z in zero_outs]
        outs = jax.block_until_ready(sharded(*args))
        bench_ns = None
        if n_bench:
            times = []
            for _ in range(n_bench):
                t0 = _time.perf_counter()
                jax.block_until_ready(sharded(*args))
                times.append(_time.perf_counter() - t0)
            bench_ns = int(min(times) * 1e9)
        y = np.asarray(outs[out_names.index("y")])
        return y, bench_ns

    _RUNNER_CACHE[t_steps] = run
    return run


def make_inputs(x, W_ih0, W_hh0, b_ih0, b_hh0, W_ih1, W_hh1, b_ih1, b_hh1,
                W_fc, b_fc):
    x = np.asarray(x, dtype=np.float32)
    t_steps = x.shape[1]
    nk = t_steps // 8 + 1
    cb = prep_weights(
        W_ih0, W_hh0, b_ih0, b_hh0, W_ih1, W_hh1, b_ih1, b_hh1, W_fc, b_fc
    )
    xt_all = np.empty((N_CORES * nk, 21, 2048), BF)
    a0_all = np.zeros((N_CORES * 117, 256), BF)
    for core in range(N_CORES):
        xc = x[core * B_LOCAL : (core + 1) * B_LOCAL]
        xt = prep_x_core(xc, t_steps)
        xt_all[core * nk : (core + 1) * nk] = xt
        a0_all[core * 117 + 100 : (core + 1) * 117] = xt[0, 4:21, 0:256]
    reps = lambda a: np.concatenate([a] * N_CORES, axis=0)
    return t_steps, {
        "xt": xt_all,
        "cblob": reps(cb),
        "a0": a0_all,
    }


def kernel(x, W_ih0, W_hh0, b_ih0, b_hh0, W_ih1, W_hh1, b_ih1, b_hh1, W_fc, b_fc,
           n_bench=0):
    global LAST_EXEC_NS
    t_steps, concat_inputs = make_inputs(
        x, W_ih0, W_hh0, b_ih0, b_hh0, W_ih1, W_hh1, b_ih1, b_hh1, W_fc, b_fc
    )
    run = _get_runner(t_steps)
    y, bench_ns = run(concat_inputs, n_bench=n_bench)
    if bench_ns is not None:
        LAST_EXEC_NS = bench_ns
    return np.ascontiguousarray(y.reshape(-1)[:, None]).astype(np.float32)

